# revision 1
# baseline (speedup 1.0000x reference)
"""EnergyNet score kernel for 8 TRN2 NeuronCores.

forward(): score_x = d/dx [ -||s|| + sum(s) + c ],  s = MLP(concat(x, t)).
Data-parallel over the batch axis. Per core (Nc samples), features-major:
  a1 = W1^T xaug            (PE, bf16)         h1 = gelu(a1), ga1 = gelu'(a1)  (ACT)
  a2 = W2^T h1              (PE)               h2, ga2                         (ACT)
  s  = h2-stationary @ W3   (PE, sample-major [128s, 64f] PSUM)
  q  = sum(s^2)  -> negr = -1/sqrt(q)          (DVE square/reduce + bit-trick rsqrt)
  srT = s * negr  (= -s*r, sample-major)       (DVE tensor_scalar per-partition scalar)
  sr  = transpose(srT)  [64f, Ns]              (PE transpose + DVE copy)
  u  = W3 @ sr   (= -W3 (s r))                 (PE)
  da2 = (u + rowsum(W3)) * ga2                 (DVE scalar_tensor_tensor; dh2 = W3@(1-s r))
  dh1 = W2 @ da2                               (PE)
  da1 = dh1 * ga1                              (DVE)
  dxT = da1-stationary @ W1x^T  [128s, 64f]    (PE)  -> dx out                 (ACT copy)
"""

import numpy as np
import ml_dtypes
from contextlib import ExitStack

import concourse.bass as bass
import concourse.mybir as mybir
import concourse.tile as tile
from concourse import bacc
from concourse.bass_utils import run_bass_kernel_spmd

dt = mybir.dt
AF = mybir.ActivationFunctionType
ALU = mybir.AluOpType

D = 64          # data dim
DA = 65         # D + 1 (concat t)
H = 256         # hidden
NCORES = 8
NS = 2048       # samples per mega-tile
NT = 512        # samples per matmul subtile
BF = ml_dtypes.bfloat16
MAGIC = 0x5F3759DF


def build(nc_samples: int, with_b3: bool):
    n_megas = nc_samples // NS
    assert n_megas * NS == nc_samples
    nc = bacc.Bacc("TRN2", target_bir_lowering=False)

    xT = nc.declare_dram_parameter("xT", [DA, nc_samples], dt.bfloat16, isOutput=False)
    w1 = nc.declare_dram_parameter("w1", [DA, H], dt.bfloat16, isOutput=False)
    w2 = nc.declare_dram_parameter("w2", [H, H], dt.bfloat16, isOutput=False)      # 2 x [128, 256] k-chunks stacked
    w2t = nc.declare_dram_parameter("w2t", [H, H], dt.bfloat16, isOutput=False)    # W2.T, same chunking
    w3 = nc.declare_dram_parameter("w3", [H, D], dt.bfloat16, isOutput=False)      # 2 x [128, 64]
    w3t = nc.declare_dram_parameter("w3t", [D, H], dt.bfloat16, isOutput=False)    # [64, 256]
    w1xt = nc.declare_dram_parameter("w1xt", [H, D], dt.bfloat16, isOutput=False)  # W1[:64].T, 2 x [128, 64]
    b1p = nc.declare_dram_parameter("b1p", [128, 2], dt.float32, isOutput=False)
    b2p = nc.declare_dram_parameter("b2p", [128, 2], dt.float32, isOutput=False)
    w3s = nc.declare_dram_parameter("w3s", [128, 2], dt.float32, isOutput=False)   # rowsum(W3) chunks
    idn = nc.declare_dram_parameter("idn", [128, 128], dt.bfloat16, isOutput=False)
    if with_b3:
        b3bc = nc.declare_dram_parameter("b3bc", [128, NS // 128 * D], dt.float32, isOutput=False)
    dx = nc.declare_dram_parameter("dx", [nc_samples, D], dt.float32, isOutput=True)
    # view: sample n = chunk*128 + p  ->  [p, chunk, f]
    dx_v = dx.rearrange("(c p) f -> p c f", p=128)

    with tile.TileContext(nc) as tc, ExitStack() as ctx:
        cst = ctx.enter_context(tc.tile_pool(name="cst", bufs=1))
        xp = ctx.enter_context(tc.tile_pool(name="xp", bufs=3))
        h1p = ctx.enter_context(tc.tile_pool(name="h1p", bufs=3))
        h2p = ctx.enter_context(tc.tile_pool(name="h2p", bufs=3))
        g1p = ctx.enter_context(tc.tile_pool(name="g1p", bufs=3))
        g2p = ctx.enter_context(tc.tile_pool(name="g2p", bufs=3))
        dap = ctx.enter_context(tc.tile_pool(name="dap", bufs=3))
        nrm = ctx.enter_context(tc.tile_pool(name="nrm", bufs=2))
        sml = ctx.enter_context(tc.tile_pool(name="sml", bufs=4))
        dxp = ctx.enter_context(tc.tile_pool(name="dxp", bufs=4))
        fps = ctx.enter_context(tc.tile_pool(name="fps", bufs=2, space="PSUM"))
        bps = ctx.enter_context(tc.tile_pool(name="bps", bufs=2, space="PSUM"))
        sps = ctx.enter_context(tc.tile_pool(name="sps", bufs=2, space="PSUM"))
        tps = ctx.enter_context(tc.tile_pool(name="tps", bufs=1, space="PSUM"))
        ops = ctx.enter_context(tc.tile_pool(name="ops", bufs=1, space="PSUM"))

        # ---- one-time constants ----
        w1_s = cst.tile([DA, H], dt.bfloat16, name="w1_s")
        nc.sync.dma_start(w1_s[:], w1[:])
        w2_s = [cst.tile([128, H], dt.bfloat16, name=f"w2_{k}") for k in range(2)]
        w2t_s = [cst.tile([128, H], dt.bfloat16, name=f"w2t_{k}") for k in range(2)]
        w3_s = [cst.tile([128, D], dt.bfloat16, name=f"w3_{k}") for k in range(2)]
        w1xt_s = [cst.tile([128, D], dt.bfloat16, name=f"w1xt_{k}") for k in range(2)]
        for k in range(2):
            nc.sync.dma_start(w2_s[k][:], w2[128 * k:128 * (k + 1), :])
            nc.sync.dma_start(w2t_s[k][:], w2t[128 * k:128 * (k + 1), :])
            nc.sync.dma_start(w3_s[k][:], w3[128 * k:128 * (k + 1), :])
            nc.sync.dma_start(w1xt_s[k][:], w1xt[128 * k:128 * (k + 1), :])
        w3t_s = cst.tile([D, H], dt.bfloat16, name="w3t_s")
        nc.sync.dma_start(w3t_s[:], w3t[:])
        b1_s = cst.tile([128, 2], dt.float32, name="b1_s")
        nc.sync.dma_start(b1_s[:], b1p[:])
        b2_s = cst.tile([128, 2], dt.float32, name="b2_s")
        nc.sync.dma_start(b2_s[:], b2p[:])
        w3s_s = cst.tile([128, 2], dt.float32, name="w3s_s")
        nc.sync.dma_start(w3s_s[:], w3s[:])
        idn_s = cst.tile([128, 128], dt.bfloat16, name="idn_s")
        nc.sync.dma_start(idn_s[:], idn[:])
        magic_s = cst.tile([128, 8], dt.int32, name="magic_s")
        nc.gpsimd.memset(magic_s[:], MAGIC)
        if with_b3:
            b3bc_s = cst.tile([128, NS // 128 * D], dt.float32, name="b3bc_s")
            nc.sync.dma_start(b3bc_s[:], b3bc[:])

        NCH = NS // 128   # 128-sample chunks per mega (16)
        HCH = NCH // 2    # chunks per half-mega (8)
        NT = 512

        for mg in range(n_megas):
            x_t = xp.tile([DA, NS], dt.bfloat16, tag="x", name="x_t")
            nc.sync.dma_start(x_t[:], xT[:, mg * NS:(mg + 1) * NS])

            h1_t = [h1p.tile([128, NS], dt.bfloat16, tag=f"h1_{m}", name=f"h1_{m}") for m in range(2)]
            g1_t = [g1p.tile([128, NS], dt.bfloat16, tag=f"g1_{m}", name=f"g1_{m}") for m in range(2)]
            h2_t = [h2p.tile([128, NS], dt.bfloat16, tag=f"h2_{m}", name=f"h2_{m}") for m in range(2)]
            g2_t = [g2p.tile([128, NS], dt.bfloat16, tag=f"g2_{m}", name=f"g2_{m}") for m in range(2)]
            sr = nrm.tile([D, NS], dt.bfloat16, tag="sr", name="sr")

            for hh in range(2):  # half-mega: fwd + norm + transpose
                s_ps = sps.tile([128, HCH * D], dt.float32, tag="s", name="s_ps")
                # software-pipelined emission: PE is in-order, so group
                # a1 matmuls of both subtiles before any a2 (which waits on ACT).
                for j in (2 * hh, 2 * hh + 1):
                    sl = slice(j * NT, (j + 1) * NT)
                    for m in range(2):
                        a1 = fps.tile([128, NT], dt.float32, tag="fwd", name="a1")
                        nc.tensor.matmul(a1[:], w1_s[:, 128 * m:128 * (m + 1)],
                                         x_t[:, sl], start=True, stop=True)
                        nc.scalar.activation(h1_t[m][:, sl], a1[:], AF.Gelu,
                                             bias=b1_s[:, m:m + 1])
                        nc.scalar.activation(g1_t[m][:, sl], a1[:], AF.Derivative_Gelu,
                                             bias=b1_s[:, m:m + 1])
                for j in (2 * hh, 2 * hh + 1):
                    sl = slice(j * NT, (j + 1) * NT)
                    for m in range(2):
                        a2 = fps.tile([128, NT], dt.float32, tag="fwd", name="a2")
                        for k in range(2):
                            nc.tensor.matmul(a2[:], w2_s[k][:, 128 * m:128 * (m + 1)],
                                             h1_t[k][:, sl], start=(k == 0), stop=(k == 1))
                        nc.scalar.activation(h2_t[m][:, sl], a2[:], AF.Gelu,
                                             bias=b2_s[:, m:m + 1])
                        nc.scalar.activation(g2_t[m][:, sl], a2[:], AF.Derivative_Gelu,
                                             bias=b2_s[:, m:m + 1])
                for j in (2 * hh, 2 * hh + 1):
                    for sc in range(4):
                        ch = (j - 2 * hh) * 4 + sc
                        csl = slice(j * NT + sc * 128, j * NT + (sc + 1) * 128)
                        for k in range(2):
                            nc.tensor.matmul(s_ps[:, ch * D:(ch + 1) * D],
                                             h2_t[k][:, csl], w3_s[k][:],
                                             start=(k == 0), stop=(k == 1))

                # norm for this half: negr = -1/||s||
                if with_b3:
                    s_in = nrm.tile([128, HCH * D], dt.float32, tag="sb3", name="s_in")
                    nc.vector.tensor_tensor(
                        s_in[:], s_ps[:], b3bc_s[:, :HCH * D], ALU.add)
                else:
                    s_in = s_ps
                sq = nrm.tile([128, HCH * D], dt.float32, tag="sq", name="sq")
                nc.scalar.activation(sq[:], s_in[:], AF.Square)
                q = sml.tile([128, HCH], dt.float32, tag="q", name="q")
                nc.vector.tensor_reduce(q[:], sq[:].rearrange("p (c f) -> p c f", f=D),
                                        mybir.AxisListType.X, ALU.add)
                yi = sml.tile([128, HCH], dt.int32, tag="yi", name="yi")
                nc.vector.tensor_scalar(yi[:], q[:].bitcast(dt.int32), 1, None,
                                        ALU.logical_shift_right)
                nc.vector.tensor_tensor(yi[:], magic_s[:], yi[:], ALU.subtract)
                y = yi[:].bitcast(dt.float32)
                qh = sml.tile([128, HCH], dt.float32, tag="qh", name="qh")
                nc.vector.tensor_scalar(qh[:], q[:], 0.5, None, ALU.mult)
                pp = sml.tile([128, HCH], dt.float32, tag="pp", name="pp")
                for it in range(3):
                    nc.vector.tensor_tensor(pp[:], y, y, ALU.mult)
                    nc.vector.tensor_tensor(pp[:], pp[:], qh[:], ALU.mult)
                    nc.vector.tensor_scalar(pp[:], pp[:], -1.5, None, ALU.add)
                    nc.vector.tensor_tensor(y, y, pp[:], ALU.mult)  # y <- -y(1.5-.5qy^2)
                nrb = nrm.tile([128, HCH * D], dt.float32, tag="nrb", name="nrb")
                nc.gpsimd.tensor_copy(
                    nrb[:].rearrange("p (c f) -> p c f", f=D),
                    y.broadcast_to([128, HCH, D]))
                srt = nrm.tile([128, HCH * D], dt.bfloat16, tag="srt", name="srt")
                nc.vector.tensor_tensor(srt[:], s_in[:], nrb[:], ALU.mult)

                sr_ps = tps.tile([D, NS // 2], dt.bfloat16, tag="srps", name="sr_ps")
                for c in range(HCH):
                    nc.tensor.transpose(sr_ps[:, c * 128:(c + 1) * 128],
                                        srt[:, c * D:(c + 1) * D], idn_s[:])
                nc.vector.tensor_copy(sr[:, hh * (NS // 2):(hh + 1) * (NS // 2)], sr_ps[:])

            # ---------------- backward ----------------
            da2_t = [dap.tile([128, NS], dt.bfloat16, tag=f"da2_{m}", name=f"da2_{m}") for m in range(2)]
            da1_t = [dap.tile([128, NS], dt.bfloat16, tag=f"da1_{m}", name=f"da1_{m}") for m in range(2)]
            for j in range(NS // NT):
                sl = slice(j * NT, (j + 1) * NT)
                for m in range(2):
                    u = bps.tile([128, NT], dt.float32, tag="bwd", name="u")
                    nc.tensor.matmul(u[:], w3t_s[:, 128 * m:128 * (m + 1)], sr[:, sl],
                                     start=True, stop=True)
                    # da2 = (u + rowsum(W3)) * ga2    [dh2 = W3@(1 - s r)]
                    nc.vector.scalar_tensor_tensor(da2_t[m][:, sl], u[:],
                                                   w3s_s[:, m:m + 1], g2_t[m][:, sl],
                                                   ALU.add, ALU.mult)
                for m in range(2):
                    dh1 = bps.tile([128, NT], dt.float32, tag="bwd", name="dh1")
                    for k in range(2):
                        nc.tensor.matmul(dh1[:], w2t_s[k][:, 128 * m:128 * (m + 1)],
                                         da2_t[k][:, sl], start=(k == 0), stop=(k == 1))
                    nc.vector.tensor_tensor(da1_t[m][:, sl], dh1[:], g1_t[m][:, sl],
                                            ALU.mult)
                dxq = ops.tile([128, 4 * D], dt.float32, tag="dxq", name="dxq")
                for sc in range(4):
                    csl = slice(j * NT + sc * 128, j * NT + (sc + 1) * 128)
                    for k in range(2):
                        nc.tensor.matmul(dxq[:, sc * D:(sc + 1) * D],
                                         da1_t[k][:, csl], w1xt_s[k][:],
                                         start=(k == 0), stop=(k == 1))
                dxs = dxp.tile([128, 4 * D], dt.float32, tag="dxs", name="dxs")
                nc.vector.tensor_copy(dxs[:], dxq[:])
                nc.gpsimd.dma_start(
                    dx_v[:, mg * NCH + j * 4:mg * NCH + (j + 1) * 4, :],
                    dxs[:].rearrange("p (c f) -> p c f", f=D))

    nc.compile()
    return nc


_CACHE = {}


def _get_nc(nc_samples, with_b3):
    key = (nc_samples, with_b3)
    if key not in _CACHE:
        _CACHE[key] = build(nc_samples, with_b3)
    return _CACHE[key]


def kernel(t, x, W1, b1, W2, b2, W3, b3, c):
    t = np.asarray(t); x = np.asarray(x)
    W1 = np.asarray(W1, np.float32); b1 = np.asarray(b1, np.float32)
    W2 = np.asarray(W2, np.float32); b2 = np.asarray(b2, np.float32)
    W3 = np.asarray(W3, np.float32); b3 = np.asarray(b3, np.float32)
    N = t.shape[0]
    npc = N // NCORES
    with_b3 = bool(np.any(b3))
    nc = _get_nc(npc, with_b3)

    xT = np.empty((DA, N), dtype=BF)
    xT[:D] = x.T
    xT[D] = t
    base = dict(
        w1=W1.astype(BF),
        w2=W2.astype(BF),
        w2t=np.ascontiguousarray(W2.T).astype(BF),
        w3=W3.astype(BF),
        w3t=np.ascontiguousarray(W3.T).astype(BF),
        w1xt=np.ascontiguousarray(W1[:D].T).astype(BF),
        b1p=np.ascontiguousarray(b1.reshape(2, 128).T),
        b2p=np.ascontiguousarray(b2.reshape(2, 128).T),
        w3s=np.ascontiguousarray(W3.sum(1).astype(np.float32).reshape(2, 128).T),
        idn=np.eye(128, dtype=BF),
    )
    if with_b3:
        base["b3bc"] = np.tile(b3, (128, NS // 128)).astype(np.float32)
    in_maps = []
    for cid in range(NCORES):
        m = dict(base)
        m["xT"] = np.ascontiguousarray(xT[:, cid * npc:(cid + 1) * npc])
        in_maps.append(m)
    res = run_bass_kernel_spmd(nc, in_maps, list(range(NCORES)))
    return np.concatenate([res.results[i]["dx"] for i in range(NCORES)], axis=0)



# revision 2
# speedup vs baseline: 1.0420x; 1.0420x over previous
"""EnergyNet score kernel v2 for 8 TRN2 NeuronCores.

forward(): score_x = d/dx [ -||s|| + sum(s) + c ],  s = MLP(concat(x, t)).
Data-parallel over batch. Per core (Nc samples), features-major, mega=2048
samples split in two column-halves (c0/c1) of 1024.

Engine plan per mega (steady state, ACT is the metronome at ~16.6us):
  ACT  16 instrs x [128,1024]: gelu/dgelu of a1,a2 (gelu first per group)
  PE   a1(4x1024) a2(4x2048) s(fp8 DR, 16x32) u(4x1024) dh1(4x2048)
       dxT(32x64), fwd(i) interleaved with bwd(i-1) to stay dense
  DVE  da2 = (u+rowsum W3)*g2, da1 = dh1*g1, sq, srt, dx copy
  Pool reduce(q), newton rsqrt, -r broadcast
  DMA  xT in, srt->sr xbar transpose, dx out
PSUM: fwd pool 2x[128,1024] (4 banks) + bwd pool 2x[128,1024] (4 banks).
"""

import numpy as np
import ml_dtypes
from contextlib import ExitStack

import concourse.bass as bass
import concourse.mybir as mybir
import concourse.tile as tile
from concourse import bacc
from concourse.bass_utils import run_bass_kernel_spmd

dt = mybir.dt
AF = mybir.ActivationFunctionType
ALU = mybir.AluOpType
PM = mybir.MatmulPerfMode

D = 64
DA = 65
H = 256
NCORES = 8
NS = 2048          # samples per mega
CH = 1024          # column-half
NCH = NS // 128    # 128-sample chunks per mega (16)
HCH = NCH // 2     # chunks per column-half (8)
BF = ml_dtypes.bfloat16
F8 = ml_dtypes.float8_e4m3
MAGIC = 0x5F3759DF


def build(nc_samples: int, with_b3: bool, with_b1: bool = False, with_b2: bool = False,
          debug: bool = False):
    n_megas = nc_samples // NS
    assert n_megas * NS == nc_samples
    nc = bacc.Bacc("TRN2", target_bir_lowering=False)
    if debug:
        dbg_h1 = nc.declare_dram_parameter("dbg_h1", [128, 2 * NS], dt.float32, isOutput=True)
        dbg_ssb = nc.declare_dram_parameter("dbg_ssb", [128, CH], dt.float32, isOutput=True)
        dbg_srt = nc.declare_dram_parameter("dbg_srt", [128, CH], dt.float32, isOutput=True)
        dbg_sr = nc.declare_dram_parameter("dbg_sr", [D, NS], dt.float32, isOutput=True)
        dbg_u = nc.declare_dram_parameter("dbg_u", [128, 2 * NS], dt.float32, isOutput=True)

    xT = nc.declare_dram_parameter("xT", [DA, nc_samples], dt.bfloat16, isOutput=False)
    w1 = nc.declare_dram_parameter("w1", [DA, H], dt.bfloat16, isOutput=False)
    w2 = nc.declare_dram_parameter("w2", [H, H], dt.bfloat16, isOutput=False)      # 2 x [128,256] k-chunks
    w2t = nc.declare_dram_parameter("w2t", [H, H], dt.bfloat16, isOutput=False)    # W2.T chunked
    w3dr = nc.declare_dram_parameter("w3dr", [128, 2 * D], dt.float8e4, isOutput=False)  # [128,2,64] k-interleaved W3
    w3t = nc.declare_dram_parameter("w3t", [D, H], dt.bfloat16, isOutput=False)    # [64, 256]
    w1xt = nc.declare_dram_parameter("w1xt", [H, D], dt.bfloat16, isOutput=False)  # W1[:64].T, 2 x [128,64]
    if with_b1 or with_b2:
        b1r = nc.declare_dram_parameter("b1r", [1, H], dt.bfloat16, isOutput=False)
        b2r = nc.declare_dram_parameter("b2r", [1, H], dt.bfloat16, isOutput=False)
    w3s = nc.declare_dram_parameter("w3s", [128, 2], dt.float32, isOutput=False)   # rowsum(W3) chunks
    idn = nc.declare_dram_parameter("idn", [128, 128], dt.bfloat16, isOutput=False)
    if with_b3:
        b3bc = nc.declare_dram_parameter("b3bc", [128, HCH * D], dt.float32, isOutput=False)
    dx = nc.declare_dram_parameter("dx", [nc_samples, D], dt.float32, isOutput=True)
    dx_v = dx.rearrange("(c p) f -> p c f", p=128)

    with tile.TileContext(nc) as tc, ExitStack() as ctx:
        cst = ctx.enter_context(tc.tile_pool(name="cst", bufs=1))
        xp = ctx.enter_context(tc.tile_pool(name="xp", bufs=3))
        hp = ctx.enter_context(tc.tile_pool(name="hp", bufs=2))
        gp = ctx.enter_context(tc.tile_pool(name="gp", bufs=3))
        dap = ctx.enter_context(tc.tile_pool(name="dap", bufs=2))
        srtp = ctx.enter_context(tc.tile_pool(name="srtp", bufs=2))
        srp = ctx.enter_context(tc.tile_pool(name="srp", bufs=3))
        nrm = ctx.enter_context(tc.tile_pool(name="nrm", bufs=2))
        dxp = ctx.enter_context(tc.tile_pool(name="dxp", bufs=2))
        fps = ctx.enter_context(tc.tile_pool(name="fps", bufs=2, space="PSUM"))
        bps = ctx.enter_context(tc.tile_pool(name="bps", bufs=2, space="PSUM"))

        xtiles = {}

        def load_x(mg):
            if mg in xtiles or mg >= n_megas:
                return
            xt = xp.tile([DA, NS], dt.bfloat16, tag="x", name="x")
            nc.sync.dma_start(xt[:], xT[:, mg * NS:(mg + 1) * NS])
            xtiles[mg] = xt

        # ---- constants (w1 + x prefetch first so compute can start early) ----
        w1_s = cst.tile([DA, H], dt.bfloat16, name="w1_s")
        nc.sync.dma_start(w1_s[:], w1[:])
        load_x(0)
        load_x(1)
        w2_s = [cst.tile([128, H], dt.bfloat16, name=f"w2_{k}") for k in range(2)]
        w2t_s = [cst.tile([128, H], dt.bfloat16, name=f"w2t_{k}") for k in range(2)]
        w1xt_s = [cst.tile([128, D], dt.bfloat16, name=f"w1xt_{k}") for k in range(2)]
        for k in range(2):
            nc.sync.dma_start(w2_s[k][:], w2[128 * k:128 * (k + 1), :])
            nc.sync.dma_start(w2t_s[k][:], w2t[128 * k:128 * (k + 1), :])
            nc.sync.dma_start(w1xt_s[k][:], w1xt[128 * k:128 * (k + 1), :])
        w3dr_s = cst.tile([128, 2 * D], dt.float8e4, name="w3dr_s")
        nc.sync.dma_start(w3dr_s[:], w3dr[:])
        w3dr_v = w3dr_s[:].rearrange("p (k f) -> p k f", k=2)
        w3t_s = cst.tile([D, H], dt.bfloat16, name="w3t_s")
        nc.sync.dma_start(w3t_s[:], w3t[:])
        if with_b1 or with_b2:
            b1_s = cst.tile([1, H], dt.bfloat16, name="b1_s")
            nc.sync.dma_start(b1_s[:], b1r[:])
            b2_s = cst.tile([1, H], dt.bfloat16, name="b2_s")
            nc.sync.dma_start(b2_s[:], b2r[:])
            ones_s = cst.tile([1, 512], dt.bfloat16, name="ones_s")
            nc.gpsimd.memset(ones_s[:], 1.0)
        w3s_s = cst.tile([128, 2], dt.float32, name="w3s_s")
        nc.sync.dma_start(w3s_s[:], w3s[:])
        idn_s = cst.tile([128, 128], dt.bfloat16, name="idn_s")
        nc.sync.dma_start(idn_s[:], idn[:])
        magic_s = cst.tile([128, HCH], dt.int32, name="magic_s")
        nc.gpsimd.memset(magic_s[:], MAGIC)
        if with_b3:
            b3_s = cst.tile([128, HCH * D], dt.float32, name="b3_s")
            nc.sync.dma_start(b3_s[:], b3bc[:])

        # per-mega live state carried across loop iterations (prev = i-1)
        prev = None

        def fwd_start(mg):
            """allocate mega-i tiles"""
            st = {}
            st["x"] = xtiles.pop(mg)
            st["h1"] = hp.tile([128, 2, NS], dt.bfloat16, tag="h1", name="h1")
            st["h2"] = hp.tile([128, 2, NS], dt.float8e4, tag="h2", name="h2")
            st["g1"] = gp.tile([128, 2, NS], dt.bfloat16, tag="g1", name="g1")
            st["g2"] = gp.tile([128, 2, NS], dt.bfloat16, tag="g2", name="g2")
            st["sr"] = srp.tile([D, NS], dt.bfloat16, tag="sr", name="sr")
            # transpose staging tile allocated early so its PSUM-bank gate
            # resolves long before the late-emitted PE transposes write it
            st["tr"] = bps.tile([128, 2 * CH], dt.bfloat16, tag="b", name="tr")
            st["s"] = [None, None]
            st["mg"] = mg
            return st

        # forward tiles are m-paired 512-sample blocks: [128, 2, 512] (2 banks)
        def pe_a1(st, b):
            bsl = slice(b * 512, (b + 1) * 512)
            a1 = fps.tile([128, 2, 512], dt.float32, tag="f", name="a1")
            for m in range(2):
                nc.tensor.matmul(a1[:, m, :],
                                 w1_s[:, 128 * m:128 * (m + 1)],
                                 st["x"][:, bsl], start=True,
                                 stop=not with_b1)
                if with_b1:
                    nc.tensor.matmul(a1[:, m, :],
                                     b1_s[:, 128 * m:128 * (m + 1)],
                                     ones_s[:], start=False, stop=True)
            st[f"a1_{b}"] = a1

        def act_l1(st, b):
            bsl = slice(b * 512, (b + 1) * 512)
            nc.scalar.activation(st["h1"][:, :, bsl], st[f"a1_{b}"][:], AF.Gelu)
            nc.scalar.activation(st["g1"][:, :, bsl], st[f"a1_{b}"][:],
                                 AF.Derivative_Gelu)

        def pe_a2(st, b):
            bsl = slice(b * 512, (b + 1) * 512)
            a2 = fps.tile([128, 2, 512], dt.float32, tag="f", name="a2")
            for m in range(2):
                for k in range(2):
                    nc.tensor.matmul(a2[:, m, :],
                                     w2_s[k][:, 128 * m:128 * (m + 1)],
                                     st["h1"][:, k, bsl],
                                     start=(k == 0), stop=(k == 1) and not with_b2)
                if with_b2:
                    nc.tensor.matmul(a2[:, m, :],
                                     b2_s[:, 128 * m:128 * (m + 1)],
                                     ones_s[:], start=False, stop=True)
            st[f"a2_{b}"] = a2

        def act_l2(st, b):
            bsl = slice(b * 512, (b + 1) * 512)
            nc.scalar.activation(st["h2"][:, :, bsl], st[f"a2_{b}"][:], AF.Gelu)
            nc.scalar.activation(st["g2"][:, :, bsl], st[f"a2_{b}"][:],
                                 AF.Derivative_Gelu)

        def pe_s(st, b):
            # 4 sample-chunks of 128 within block b, fp8 DoubleRow
            c = b // 2
            if st["s"][c] is None:
                st["s"][c] = bps.tile([128, CH], dt.float32, tag="b", name="s")
            s = st["s"][c]
            for ch4 in range(4):
                ch = (b % 2) * 4 + ch4
                nc.tensor.matmul(s[:, ch * D:(ch + 1) * D],
                                 st["h2"][:, :, b * 512 + ch4 * 128:b * 512 + (ch4 + 1) * 128],
                                 w3dr_v,
                                 start=True, stop=True, perf_mode=PM.DoubleRow)

        def norm_a(st, c):
            """s -> q -> rsqrt -> srt (sample-major bf16)"""
            s = st["s"][c]
            # evacuate PSUM once; all remaining elementwise runs SBUF-side
            s_sb = nrm.tile([128, HCH * D], dt.float32, tag="ssb", name="s_sb")
            if with_b3:
                nc.vector.tensor_tensor(s_sb[:], s[:, :HCH * D], b3_s[:], ALU.add)
            else:
                nc.vector.tensor_copy(s_sb[:], s[:, :HCH * D])
            s_cf = s_sb[:].rearrange("p (c f) -> p c f", f=D)
            sq = nrm.tile([128, HCH * D], dt.float32, tag="sq", name="sq")
            nc.gpsimd.tensor_tensor(
                sq[:].rearrange("p (c f) -> p c f", f=D), s_cf, s_cf, ALU.mult)
            q = nrm.tile([128, HCH], dt.float32, tag="q", name="q")
            nc.vector.tensor_reduce(q[:], sq[:].rearrange("p (c f) -> p c f", f=D),
                                    mybir.AxisListType.X, ALU.add)
            # magic rsqrt + 2 newton iters (tensor_tensor on Pool, tensor_scalar on DVE)
            yi = nrm.tile([128, HCH], dt.int32, tag="yi", name="yi")
            nc.vector.tensor_scalar(yi[:], q[:].bitcast(dt.int32), 1, None,
                                    ALU.logical_shift_right)
            nc.gpsimd.tensor_tensor(yi[:], magic_s[:], yi[:], ALU.subtract)
            y = yi[:].bitcast(dt.float32)
            pp = nrm.tile([128, HCH], dt.float32, tag="pp", name="pp")
            for _ in range(2):
                nc.gpsimd.tensor_tensor(pp[:], y, y, ALU.mult)
                nc.gpsimd.tensor_tensor(pp[:], pp[:], q[:], ALU.mult)
                nc.vector.tensor_scalar(pp[:], pp[:], -0.5, 1.5, ALU.mult, ALU.add)
                nc.gpsimd.tensor_tensor(y, y, pp[:], ALU.mult)
            # nrb = +r broadcast to [128, c, f]; the sign of -s*r is
            # absorbed by da2 = (u - rowsum(W3)) * g2 and negated w1xt
            nrb = nrm.tile([128, HCH * D], dt.float32, tag="nrb", name="nrb")
            nrb_cf = nrb[:].rearrange("p (c f) -> p c f", f=D)
            nc.gpsimd.tensor_copy(nrb_cf, y.broadcast_to([128, HCH, D]))
            # srt[p, c*D + f] = s[p, c, f] * r   (sample-major, natural layout)
            srt = srtp.tile([128, HCH * D], dt.bfloat16, tag="srt", name="srt")
            nc.gpsimd.tensor_tensor(
                srt[:].rearrange("p (c f) -> p c f", f=D), s_cf, nrb_cf, ALU.mult)
            st[f"srt_{c}"] = srt
            if debug and st["mg"] == 0 and c == 0:
                nc.gpsimd.dma_start(dbg_ssb[:, :HCH * D], s_sb[:])
                tmp2 = nrm.tile([128, HCH * D], dt.float32, tag="dbgsrt", name="dbgsrt")
                nc.vector.tensor_copy(tmp2[:], srt[:])
                nc.gpsimd.dma_start(dbg_srt[:, :HCH * D], tmp2[:])

        def norm_b(st, c):
            """PE-transpose srt chunks -> PSUM bf16 -> DVE copy to sr SBUF."""
            srt = st[f"srt_{c}"]
            tr = st["tr"]
            for ch in range(HCH):
                nc.tensor.transpose(tr[:D, c * CH + ch * 128:c * CH + (ch + 1) * 128],
                                    srt[:, ch * D:(ch + 1) * D], idn_s[:])
            nc.vector.tensor_copy(
                st["sr"][:D, c * CH:(c + 1) * CH].bitcast(dt.int32),
                tr[:D, c * CH:(c + 1) * CH].bitcast(dt.int32))

        def pe_u(st, c):
            for m in range(2):
                u = bps.tile([128, CH], dt.float32, tag="b", name="u")
                for j in range(2):
                    nc.tensor.matmul(u[:, j * 512:(j + 1) * 512],
                                     w3t_s[:, 128 * m:128 * (m + 1)],
                                     st["sr"][:, c * CH + j * 512:c * CH + (j + 1) * 512],
                                     start=True, stop=True)
                st[f"u_{m}_{c}"] = u

        def dbg_dump(st):
            if not debug or st["mg"] != 0:
                return
            tmp = nrm.tile([128, 2 * NS], dt.float32, tag="dbgh1", name="dbgh1")
            nc.vector.tensor_copy(tmp[:], st["h1"][:, :, :].rearrange("p m n -> p (m n)"))
            nc.gpsimd.dma_start(dbg_h1[:, :], tmp[:])
            tmp3 = nrm.tile([D, NS], dt.float32, tag="dbgsr", name="dbgsr")
            nc.vector.tensor_copy(tmp3[:], st["sr"][:, :])
            nc.gpsimd.dma_start(dbg_sr[:, :], tmp3[:])

        def dbg_dump_u(st):
            if not debug or st["mg"] != 0:
                return
            tmp = nrm.tile([128, 2 * NS], dt.float32, tag="dbgu", name="dbgu")
            for m in range(2):
                for c in range(2):
                    nc.vector.tensor_copy(tmp[:, (m * 2 + c) * CH:(m * 2 + c + 1) * CH],
                                          st[f"u_{m}_{c}"][:])
                    nc.gpsimd.dma_start(dbg_u[:, (m * 2 + c) * CH:(m * 2 + c + 1) * CH],
                                        tmp[:, (m * 2 + c) * CH:(m * 2 + c + 1) * CH])

        def dve_da2(st, c):
            if f"da2" not in st:
                st["da2"] = dap.tile([128, 2, NS], dt.bfloat16, tag="da2", name="da2")
            csl = slice(c * CH, (c + 1) * CH)
            for m in range(2):
                # da2 = (u - rowsum(W3)) * g2 = -(true da2); the sign is
                # restored by the negated w1xt in the final matmul
                nc.vector.scalar_tensor_tensor(
                    st["da2"][:, m, csl], st[f"u_{m}_{c}"][:],
                    w3s_s[:, m:m + 1], st["g2"][:, m, csl], ALU.subtract, ALU.mult)

        def pe_dh1(st, c):
            for m in range(2):
                dh = bps.tile([128, CH], dt.float32, tag="b", name="dh")
                for j in range(2):
                    sl = slice(c * CH + j * 512, c * CH + (j + 1) * 512)
                    for k in range(2):
                        nc.tensor.matmul(dh[:, j * 512:(j + 1) * 512],
                                         w2t_s[k][:, 128 * m:128 * (m + 1)],
                                         st["da2"][:, k, sl],
                                         start=(k == 0), stop=(k == 1))
                st[f"dh_{m}_{c}"] = dh

        def dve_da1(st, c):
            if "da1" not in st:
                st["da1"] = dap.tile([128, 2, NS], dt.bfloat16, tag="da1", name="da1")
            csl = slice(c * CH, (c + 1) * CH)
            for m in range(2):
                nc.vector.tensor_tensor(
                    st["da1"][:, m, csl], st[f"dh_{m}_{c}"][:],
                    st["g1"][:, m, csl], ALU.mult)

        def pe_dxt(st):
            dxq = bps.tile([128, CH], dt.float32, tag="b", name="dxq")
            for ch in range(NCH):
                for k in range(2):
                    nc.tensor.matmul(dxq[:, ch * D:(ch + 1) * D],
                                     st["da1"][:, k, ch * 128:(ch + 1) * 128],
                                     w1xt_s[k][:],
                                     start=(k == 0), stop=(k == 1))
            st["dxq"] = dxq

        def dx_out(st):
            dxs = dxp.tile([128, NCH * D], dt.float32, tag="dxs", name="dxs")
            nc.vector.tensor_copy(dxs[:], st["dxq"][:])
            mg = st["mg"]
            nc.gpsimd.dma_start(
                dx_v[:, mg * NCH:(mg + 1) * NCH, :],
                dxs[:].rearrange("p (c f) -> p c f", f=D))

        prev2 = None
        for i in range(n_megas + 2):
            load_x(i + 2)
            cur = fwd_start(i) if i < n_megas else None
            pv = prev2  # backward runs with a 2-mega lag

            if cur:
                pe_a1(cur, 0)
                act_l1(cur, 0)
            if prev:
                norm_b(prev, 1)
            if pv:
                pe_u(pv, 0)
                dve_da2(pv, 0)
            if cur:
                pe_a1(cur, 1)
                act_l1(cur, 1)
            if pv:
                pe_u(pv, 1)
                dve_da2(pv, 1)
            if cur:
                pe_a1(cur, 2)
                act_l1(cur, 2)
            if pv:
                pe_dh1(pv, 0)
                dve_da1(pv, 0)
            if cur:
                pe_a1(cur, 3)
                act_l1(cur, 3)
            if pv:
                pe_dh1(pv, 1)
                dve_da1(pv, 1)
            if cur:
                pe_a2(cur, 0)
                act_l2(cur, 0)
                pe_a2(cur, 1)
                act_l2(cur, 1)
                pe_s(cur, 0)
                pe_a2(cur, 2)
                act_l2(cur, 2)
                pe_s(cur, 1)
                norm_a(cur, 0)
            if pv:
                pe_dxt(pv)
                dx_out(pv)
            if cur:
                pe_a2(cur, 3)
                act_l2(cur, 3)
                pe_s(cur, 2)
                pe_s(cur, 3)
                norm_a(cur, 1)
                norm_b(cur, 0)
                dbg_dump(cur)
            prev2 = prev
            prev = cur

    nc.compile()
    return nc


_CACHE = {}


def _get_nc(nc_samples, with_b3, with_b1=False, with_b2=False):
    key = (nc_samples, with_b3, with_b1, with_b2)
    if key not in _CACHE:
        _CACHE[key] = build(nc_samples, with_b3, with_b1, with_b2)
    return _CACHE[key]


def make_inputs(t, x, W1, b1, W2, b2, W3, b3):
    N = t.shape[0]
    xT = np.empty((DA, N), dtype=BF)
    xT[:D] = x.T
    xT[D] = t
    w3i = np.ascontiguousarray(
        W3.reshape(2, 128, D).transpose(1, 0, 2).reshape(128, 2 * D)).astype(F8)
    base = dict(
        w1=W1.astype(BF),
        w2=W2.astype(BF),
        w2t=np.ascontiguousarray(W2.T).astype(BF),
        w3dr=w3i,
        w3t=np.ascontiguousarray(W3.T).astype(BF),
        # the backward chain carries a global minus sign (da2 = (u - rowsum)*g2);
        # negating W1x here flips dx back to the correct sign
        w1xt=np.ascontiguousarray(-W1[:D].T).astype(BF),
        w3s=np.ascontiguousarray(W3.sum(1).astype(np.float32).reshape(2, 128).T),
        idn=np.eye(128, dtype=BF),
    )
    if np.any(b1) or np.any(b2):
        base["b1r"] = b1.reshape(1, H).astype(BF)
        base["b2r"] = b2.reshape(1, H).astype(BF)
    return xT, base


def kernel(t, x, W1, b1, W2, b2, W3, b3, c):
    t = np.asarray(t, np.float32); x = np.asarray(x, np.float32)
    W1 = np.asarray(W1, np.float32); b1 = np.asarray(b1, np.float32)
    W2 = np.asarray(W2, np.float32); b2 = np.asarray(b2, np.float32)
    W3 = np.asarray(W3, np.float32); b3 = np.asarray(b3, np.float32)
    N = t.shape[0]
    npc = N // NCORES
    with_b3 = bool(np.any(b3))
    with_b1 = bool(np.any(b1)); with_b2 = bool(np.any(b2))
    nc = _get_nc(npc, with_b3, with_b1, with_b2)
    xT, base = make_inputs(t, x, W1, b1, W2, b2, W3, b3)
    if with_b3:
        base["b3bc"] = np.tile(b3, (128, HCH)).astype(np.float32)
    in_maps = []
    for cid in range(NCORES):
        m = dict(base)
        m["xT"] = np.ascontiguousarray(xT[:, cid * npc:(cid + 1) * npc])
        in_maps.append(m)
    res = run_bass_kernel_spmd(nc, in_maps, list(range(NCORES)))
    return np.concatenate([res.results[i]["dx"] for i in range(NCORES)], axis=0)
